# revision 21
# baseline (speedup 1.0000x reference)
"""DinoV2 detection loss on 8 Trainium2 NeuronCores (Bass/Tile).

Reference computation (per batch sample b; B=128, Q=2048, C=365, T=50):
  dist[q, t] = sum_d |pred_boxes[b,q,d] - target_boxes[b,t,d]|
  closest[t] = argmin_q dist[q, t]
  class_targets = scatter(zeros(Q), closest, labels)     (last write wins)
  loss_ce  = weighted CE over all Q rows (background cls 0 weight 0.1)
  loss_bbox = mean_t,d |pred_boxes[closest[t]] - target_boxes[t]|
  out = mean_b(2*loss_ce + 5*loss_bbox)

Sharding: data-parallel over B; each core handles 16 samples and emits
16 per-sample losses; host averages 128 values.

Per-core device algorithm (samples processed in 8 pairs of 2, laid out on
100 partitions = 2 x 50 targets):
  - Main CE pass over a host-transposed bf16 copy of the logits
    [sample, class, query]: ACT exponentiates two samples per op
    ([128, 4096] per op, bf16 out), PE reduces classes via per-sample
    selector-column matmuls that accumulate each sample's row sums into
    a single persistent PSUM tile [16, 4x512] (f32), ACT takes Ln
    directly from PSUM at the end -> row LSE for all 16 samples at once.
    S_b = sum_q (LSE - logit[...,0]) via one DVE subtract + reduce.
  - Distances: PE matmul trick gives diff[t,(q,d)] = pb[q,d] - tb[t,d]
    (contraction over indicator/value rows; boxes split hi/lo into two
    bf16 components for near-fp32 accuracy; two block-diagonal K=12
    matmuls per 256-query chunk cover all 4 box dims), DVE reduces
    |diff| over d via one XY-axis abs-add reduce per chunk, then
    reduce-min + max_index give (min dist, argmin).
  - Duplicate-match resolution ("last write wins") via an indirect-DMA
    scatter of the target index to dram slot [matched row] followed by a
    gather readback on the same FIFO DMA queue; mask = (readback == t).
  - Matched corrections: indirect-DMA gather of the 50 matched logit
    rows per sample from the row-major f32 logits, exp+accum for their
    LSE; the target-class logit comes from a second single-element
    indirect gather at flat offset row*365+label.
  - GPSIMD (Pool) engine runs all small glue ops (casts, broadcasts,
    masks) to keep DVE on the distance-reduction critical path.
"""

import numpy as np

B, Q, C, T = 128, 2048, 365, 50
NCORES = 8
NLOC = B // NCORES          # 16 samples per core
NPAIR = NLOC // 2           # 8 pairs
P2 = 2 * T                  # 100 partitions per pair tile
QCH = 256                   # dist matmul q-chunk (x2 box dims = 512 cols)
NQC = Q // QCH              # 8
W_BG = float(np.float32(0.1))
DEN0 = float(np.float32(0.1) * 2048)   # background weight sum

_CACHE = {}


def _build_nc():
    import concourse.bacc as bacc
    import concourse.bass as bass
    import concourse.mybir as mybir
    import concourse.tile as tile

    f32 = mybir.dt.float32
    bf16 = mybir.dt.bfloat16
    i32 = mybir.dt.int32
    Alu = mybir.AluOpType
    Act = mybir.ActivationFunctionType
    Ax = mybir.AxisListType

    nc = bacc.Bacc("TRN2", target_bir_lowering=False, debug=False)

    # row-major f32 logits: read by the matched-row / matched-element gathers
    logits = nc.dram_tensor("logits", [NLOC * Q, C], f32, kind="ExternalInput")
    # transposed bf16 logits for the bulk CE pass, repacked as
    # [sample, class-chunk, q-half, class-in-chunk, q-in-half]: each
    # (sample, chunk) block is one contiguous 512KB region whose DMA
    # partition stride is 2KB -- both properties are needed for the
    # descriptor splitter to spread the transfer across all 16 SDMA
    # engines. Classes padded 365->384 with -30 (exp ~ 0).
    f8 = mybir.dt.float8e4
    logits_q = nc.dram_tensor(
        "logits_q", [NPAIR, 3, 2, 2, 128, Q // 2], f8, kind="ExternalInput"
    )
    l0h = nc.dram_tensor("l0h", [NLOC, Q], f32, kind="ExternalInput")
    # block-diagonal hi/lo bf16 operands of the box-diff matmuls:
    # [pair, dim-pair h, 12 rows, Q, 2]: two K=12 matmuls per q-chunk
    # produce diff for d = 2h+dd at psum col (q, dd).
    mmrhs = nc.dram_tensor("mmrhs", [NPAIR, 2, 12, Q, 2], bf16, kind="ExternalInput")
    mmlhs = nc.dram_tensor("mmlhs", [NPAIR, 2, 12, P2], bf16, kind="ExternalInput")
    labels = nc.dram_tensor("labels", [NLOC, T], f32, kind="ExternalInput")
    # esel[c, s, j] = (j == s): selector columns for per-sample row sums
    esel = nc.dram_tensor("esel", [128, 16, 16], bf16, kind="ExternalInput")
    halfoff = nc.dram_tensor("halfoff", [P2, 1], f32, kind="ExternalInput")
    tvec = nc.dram_tensor("tvec", [P2, 1], f32, kind="ExternalInput")
    blockhalf = nc.dram_tensor("blockhalf", [P2, 2], f32, kind="ExternalInput")
    # scratch for scatter/readback duplicate detection
    dupbuf = nc.dram_tensor("dupbuf", [NLOC * Q, 1], f32, kind="Internal")
    loss16 = nc.dram_tensor("loss16", [2, NPAIR], f32, kind="ExternalOutput")

    with tile.TileContext(nc) as tc:
        with (
            tc.tile_pool(name="const", bufs=1) as cpool,
            tc.tile_pool(name="logits", bufs=4) as lpool,
            tc.tile_pool(name="expbf", bufs=3) as epool,
            tc.tile_pool(name="scr", bufs=2) as spool,
            tc.tile_pool(name="acc", bufs=1) as apool,
            tc.tile_pool(name="pair", bufs=3) as ppool,
            tc.tile_pool(name="dram", bufs=1, space="DRAM") as dpool,
            tc.tile_pool(name="psd", bufs=2, space="PSUM") as psd,
            tc.tile_pool(name="ceps", bufs=1, space="PSUM") as pce,
        ):
            # ---- prefetch pair-0 box operands, then the first sample-pair ----
            def emit_pair_dma(p):
                rhs_t = ppool.tile([12, 2, Q, 2], bf16, tag="rhs_t")
                nc.sync.dma_start(out=rhs_t[:, 0], in_=mmrhs.ap()[p, 0])
                nc.gpsimd.dma_start(out=rhs_t[:, 1], in_=mmrhs.ap()[p, 1])
                lhs_t = ppool.tile([12, 2, P2], bf16, tag="lhs_t")
                nc.gpsimd.dma_start(
                    out=lhs_t[:],
                    in_=mmlhs.ap()[p, :, :, :].rearrange("h k t -> k h t"),
                )
                dist = ppool.tile([P2, Q], f32, tag="dist")
                return rhs_t, lhs_t, dist

            pair_tiles = {0: emit_pair_dma(0)}
            ch_tiles = {}
            for cc in range(3):
                ch = lpool.tile([128, 2, 2, Q // 2], f8, tag="chunk")
                for si in range(2):
                    nc.sync.dma_start(
                        out=ch[:, si, :, :],
                        in_=logits_q.ap()[0, cc, si].rearrange("qh c l -> c qh l"),
                    )
                ch_tiles[(0, cc)] = ch
            esel_sb = cpool.tile([128, 16, 16], bf16, tag="esel")
            nc.sync.dma_start(out=esel_sb[:], in_=esel.ap())

            # ---- constants into SBUF ----
            hoff_sb = cpool.tile([P2, 1], f32, tag="hoff")
            nc.gpsimd.dma_start(out=hoff_sb[:], in_=halfoff.ap())
            tvec_sb = cpool.tile([P2, 1], f32, tag="tvec")
            nc.gpsimd.dma_start(out=tvec_sb[:], in_=tvec.ap())
            # labels -> [100, 8]: partition (h*50+t), col p holds labels[2p+h, t]
            lab_sb = cpool.tile([P2, NPAIR], f32, tag="lab")
            lab_src = bass.AP(
                tensor=labels, offset=0, ap=[[T, 2], [1, T], [2 * T, NPAIR]]
            )
            nc.gpsimd.dma_start(out=lab_sb[:], in_=lab_src)
            bh_sb = cpool.tile([P2, 2], f32, tag="bh")
            nc.gpsimd.dma_start(out=bh_sb[:], in_=blockhalf.ap())
            # zero-fill the duplicate-detection scratch once (on the same
            # FIFO queue as the later scatters)
            zf = cpool.tile([128, NLOC * Q // 128], f32, tag="zf")
            nc.gpsimd.memset(zf[:], 0.0)
            nc.gpsimd.dma_start(
                out=dupbuf.ap().rearrange("(a b) o -> a (b o)", a=128),
                in_=zf[:],
            )

            # ---- accumulators ----
            l0_all = apool.tile([NLOC, Q], f32, tag="l0")
            lse_all = apool.tile([NLOC, Q], f32, tag="lse")
            mind_all = apool.tile([P2, NPAIR], f32, tag="mind")
            mask_all = apool.tile([P2, NPAIR], f32, tag="mask")
            sume_all = apool.tile([P2, NPAIR], f32, tag="sume")
            ly_all = apool.tile([P2, NPAIR], f32, tag="ly")
            l0m_all = apool.tile([P2, NPAIR], f32, tag="l0m")
            rb_all = apool.tile([P2, NPAIR], f32, tag="rball")

            # persistent CE row-sum accumulator: bank g holds queries
            # [512g, 512(g+1)) for all 16 samples
            ce_ps = pce.tile([NLOC, 4, 512], f32, tag="ceps")

            def emit_pair_dma(p):
                rhs_t = ppool.tile([12, 2, Q, 2], bf16, tag="rhs_t")
                nc.sync.dma_start(out=rhs_t[:, 0], in_=mmrhs.ap()[p, 0])
                nc.gpsimd.dma_start(out=rhs_t[:, 1], in_=mmrhs.ap()[p, 1])
                lhs_t = ppool.tile([12, 2, P2], bf16, tag="lhs_t")
                nc.gpsimd.dma_start(
                    out=lhs_t[:],
                    in_=mmlhs.ap()[p, :, :, :].rearrange("h k t -> k h t"),
                )
                dist = ppool.tile([P2, Q], f32, tag="dist")
                return rhs_t, lhs_t, dist

            def emit_pair_chunk(pt, qc):
                rhs_t, lhs_t, dist = pt
                ps = psd.tile([P2, 2, QCH, 2], f32, tag="psd")
                for h in range(2):
                    nc.tensor.matmul(
                        out=ps[:, h, :, :],
                        lhsT=lhs_t[:, h, :],
                        rhs=rhs_t[:, h, qc * QCH : (qc + 1) * QCH, :],
                        start=True,
                        stop=True,
                    )
                nc.vector.tensor_reduce(
                    out=dist[:, qc * QCH : (qc + 1) * QCH],
                    in_=ps[:].rearrange("p h q d -> p q h d"),
                    axis=Ax.XY,
                    op=Alu.add,
                    apply_absolute_value=True,
                )

            def emit_spair_cc(sp, cc):
                # two samples (2sp, 2sp+1) per exp op; selector-column
                # matmuls accumulate each sample's class sums into ce_ps
                if (sp, cc) in ch_tiles:
                    ch = ch_tiles.pop((sp, cc))
                else:
                    ch = lpool.tile([128, 2, 2, Q // 2], f8, tag="chunk")
                    for si in range(2):
                        nc.sync.dma_start(
                            out=ch[:, si, :, :],
                            in_=logits_q.ap()[sp, cc, si].rearrange(
                                "qh c l -> c qh l"
                            ),
                        )
                eb = epool.tile([128, 2, 2, Q // 2], bf16, tag="expbf")
                nc.scalar.activation(eb[:], ch[:], Act.Exp)
                for si in range(2):
                    s = 2 * sp + si
                    for g in range(4):
                        qh, lh = g // 2, g % 2
                        nc.tensor.matmul(
                            out=ce_ps[:, g, :],
                            lhsT=esel_sb[:, s, :],
                            rhs=eb[:, si, qh, lh * 512 : (lh + 1) * 512],
                            start=(sp == 0 and cc == 0 and si == 0),
                            stop=(sp == NPAIR - 1 and cc == 2 and si == 1),
                        )

            def emit_pair_finalize(p, pt):
                _, _, dist = pt
                nc.vector.tensor_reduce(
                    out=mind_all[:, p : p + 1], in_=dist[:], axis=Ax.X, op=Alu.min
                )
                mind8 = ppool.tile([P2, 8], f32, tag="mind8")
                nc.vector.tensor_copy(
                    out=mind8[:], in_=mind_all[:, p : p + 1].to_broadcast([P2, 8])
                )
                idxu = ppool.tile([P2, 8], mybir.dt.uint32, tag="idxu")
                nc.vector.max_index(out=idxu[:], in_max=mind8[:], in_values=dist[:])
                idxf = ppool.tile([P2, 1], f32, tag="idxf")
                nc.gpsimd.tensor_copy(out=idxf[:], in_=idxu[:, 0:1])
                rowf = ppool.tile([P2, 1], f32, tag="rowf")
                nc.gpsimd.tensor_scalar(
                    rowf[:],
                    idxf[:],
                    hoff_sb[:],
                    float(p * 2 * Q),
                    op0=Alu.add,
                    op1=Alu.add,
                )
                rowi = ppool.tile([P2, 1], i32, tag="rowi")
                nc.gpsimd.tensor_copy(out=rowi[:], in_=rowf[:])
                # flat offset of the target-class logit: row*365 + label
                rowlyf = ppool.tile([P2, 1], f32, tag="rowlyf")
                nc.gpsimd.tensor_scalar(
                    rowlyf[:],
                    rowf[:],
                    float(C),
                    lab_sb[:, p : p + 1],
                    op0=Alu.mult,
                    op1=Alu.add,
                )
                rowlyi = ppool.tile([P2, 1], i32, tag="rowlyi")
                nc.gpsimd.tensor_copy(out=rowlyi[:], in_=rowlyf[:])

                # duplicate detection: scatter t -> dupbuf[row[t]] then read
                # back on the same FIFO queue; last write wins, matching the
                # reference scatter semantics.
                nc.gpsimd.indirect_dma_start(
                    out=dupbuf.ap(),
                    out_offset=bass.IndirectOffsetOnAxis(ap=rowi[:, 0:1], axis=0),
                    in_=tvec_sb[:, 0:1],
                    in_offset=None,
                )
                nc.gpsimd.indirect_dma_start(
                    out=rb_all[:, p : p + 1],
                    out_offset=None,
                    in_=dupbuf.ap(),
                    in_offset=bass.IndirectOffsetOnAxis(ap=rowi[:, 0:1], axis=0),
                )

                # gather matched logit rows (row-major f32 copy)
                rows_sb = ppool.tile([P2, C], f32, tag="rows")
                nc.gpsimd.indirect_dma_start(
                    out=rows_sb[:],
                    out_offset=None,
                    in_=logits.ap(),
                    in_offset=bass.IndirectOffsetOnAxis(ap=rowi[:, 0:1], axis=0),
                )
                # target-class logit via single-element gather on flat view
                flat = bass.AP(tensor=logits, offset=0, ap=[[1, NLOC * Q * C], [1, 1]])
                nc.gpsimd.indirect_dma_start(
                    out=ly_all[:, p : p + 1],
                    out_offset=None,
                    in_=flat,
                    in_offset=bass.IndirectOffsetOnAxis(ap=rowlyi[:, 0:1], axis=0),
                )
                nc.gpsimd.tensor_copy(
                    out=l0m_all[:, p : p + 1], in_=rows_sb[:, 0:1]
                )
                return rows_sb

            def emit_matched(p, rows_sb):
                scr2 = spool.tile([P2, C], f32, tag="expdump")
                nc.scalar.activation(
                    scr2[:],
                    rows_sb[:],
                    Act.Exp,
                    accum_out=sume_all[:, p : p + 1],
                )

            # main pass: pair p's box-diff chunks interleave with sample-pair
            # p's CE work (dist chunks lead each cc section so PE has ready
            # work while exp cooks); pair DMAs prefetch one spair ahead;
            # matched-row work trails by one pair.
            rows_tiles = {}
            # l0 loads after the critical head DMAs
            nc.gpsimd.dma_start(out=l0_all[:], in_=l0h.ap())
            CHSPLIT = [(0, 4), (4, 8), (8, 8)]
            for sp in range(NPAIR):
                if sp + 1 < NPAIR:
                    pair_tiles[sp + 1] = emit_pair_dma(sp + 1)
                pt = pair_tiles[sp]
                for cc in range(3):
                    for qc in range(*CHSPLIT[cc]):
                        emit_pair_chunk(pt, qc)
                    emit_spair_cc(sp, cc)
                    if cc == 1:
                        # finalize overlaps the cc2 CE section
                        rows_tiles[sp] = emit_pair_finalize(sp, pt)
                        if sp > 0:
                            emit_matched(sp - 1, rows_tiles[sp - 1])
                if sp == NPAIR - 1:
                    emit_matched(sp, rows_tiles[sp])

            # ---- main CE reduction: S_b = sum_q (LSE - l0) ----
            nc.scalar.activation(
                lse_all[:],
                ce_ps[:].rearrange("s g q -> s (g q)"),
                Act.Ln,
            )
            diff = apool.tile([NLOC, Q], f32, tag="diff")
            nc.vector.tensor_sub(diff[:], lse_all[:], l0_all[:])
            s16 = apool.tile([NLOC, 1], f32, tag="s16")
            nc.vector.tensor_reduce(
                out=s16[:], in_=diff[:], axis=Ax.X, op=Alu.add
            )
            # [16,1] -> [2,8] via DRAM bounce: s = 2p + h
            s16d = dpool.tile([1, NLOC], f32, tag="s16d")
            nc.gpsimd.dma_start(out=s16d[:], in_=s16[:])
            s2 = apool.tile([2, NPAIR], f32, tag="s2")
            nc.gpsimd.dma_start(
                out=s2[:], in_=s16d[:].rearrange("o (pp h) -> o h pp", h=2)
            )

            # ---- matched-term assembly ----
            nc.vector.tensor_tensor(
                out=mask_all[:],
                in0=rb_all[:],
                in1=tvec_sb[:].to_broadcast([P2, NPAIR]),
                op=Alu.is_equal,
            )
            lsem = apool.tile([P2, NPAIR], f32, tag="lsem")
            nc.scalar.activation(lsem[:], sume_all[:], Act.Ln)
            wy = apool.tile([P2, NPAIR], f32, tag="wy")
            # wy = 1 - 0.9*(label==0)
            nc.vector.tensor_scalar(
                wy[:], lab_sb[:], 0.0, None, op0=Alu.is_equal
            )
            nc.gpsimd.tensor_scalar(
                wy[:], wy[:], -(1.0 - W_BG), 1.0, op0=Alu.mult, op1=Alu.add
            )
            nllm = apool.tile([P2, NPAIR], f32, tag="nllm")
            nc.vector.tensor_sub(nllm[:], lsem[:], ly_all[:])
            stack3 = apool.tile([P2, 3 * NPAIR], f32, tag="stack3")
            corr = stack3[:, 0:NPAIR]
            nc.vector.tensor_mul(corr, wy[:], nllm[:])
            t2 = apool.tile([P2, NPAIR], f32, tag="t2")
            nc.gpsimd.tensor_scalar(
                t2[:], lsem[:], -W_BG, None, op0=Alu.mult
            )
            nc.vector.tensor_add(corr, corr, t2[:])
            nc.gpsimd.tensor_scalar(
                t2[:], l0m_all[:], W_BG, None, op0=Alu.mult
            )
            nc.vector.tensor_add(corr, corr, t2[:])
            nc.vector.tensor_mul(corr, corr, mask_all[:])
            wadd = stack3[:, NPAIR : 2 * NPAIR]
            nc.gpsimd.tensor_scalar(
                wadd, wy[:], -W_BG, None, op0=Alu.add
            )
            nc.vector.tensor_mul(wadd, wadd, mask_all[:])
            nc.gpsimd.tensor_copy(out=stack3[:, 2 * NPAIR :], in_=mind_all[:])

            # final per-sample combine: reuse bank 0 of the (now read) CE
            # psum tile for the [2, 24] block-sum matmul
            ps_c = ce_ps[0:2, 0, 0:24]
            nc.tensor.matmul(
                out=ps_c, lhsT=bh_sb[:], rhs=stack3[:], start=True, stop=True
            )
            num = apool.tile([2, NPAIR], f32, tag="num")
            nc.vector.tensor_scalar(num[:], s2[:], W_BG, None, op0=Alu.mult)
            nc.vector.tensor_add(num[:], num[:], ps_c[:, 0:NPAIR])
            den = apool.tile([2, NPAIR], f32, tag="den")
            nc.vector.tensor_scalar(
                den[:], ps_c[:, NPAIR : 2 * NPAIR], DEN0, None, op0=Alu.add
            )
            rden = apool.tile([2, NPAIR], f32, tag="rden")
            nc.vector.reciprocal(rden[:], den[:])
            lce = apool.tile([2, NPAIR], f32, tag="lce")
            nc.vector.tensor_mul(lce[:], num[:], rden[:])
            nc.vector.tensor_scalar(lce[:], lce[:], 2.0, None, op0=Alu.mult)
            bbox = apool.tile([2, NPAIR], f32, tag="bbox")
            nc.vector.tensor_scalar(
                bbox[:], ps_c[:, 2 * NPAIR :], 5.0 / (T * 4), None, op0=Alu.mult
            )
            out_sb = apool.tile([2, NPAIR], f32, tag="out")
            nc.vector.tensor_add(out_sb[:], lce[:], bbox[:])
            nc.sync.dma_start(out=loss16.ap(), in_=out_sb[:])

    nc.compile()
    return nc


def get_nc():
    if "nc" not in _CACHE:
        _CACHE["nc"] = _build_nc()
    return _CACHE["nc"]


def _consts():
    import ml_dtypes

    halfoff = ((np.arange(P2) >= T) * Q).astype(np.float32)[:, None]
    tvec = np.arange(P2, dtype=np.float32)[:, None]
    esel = np.zeros((128, 16, 16), ml_dtypes.bfloat16)
    for s in range(16):
        esel[:, s, s] = 1.0
    blockhalf = np.zeros((P2, 2), np.float32)
    blockhalf[:T, 0] = 1.0
    blockhalf[T:, 1] = 1.0
    return {
        "halfoff": halfoff,
        "tvec": tvec,
        "esel": esel,
        "blockhalf": blockhalf,
    }


def _bf16_split(x):
    import ml_dtypes

    hi = x.astype(ml_dtypes.bfloat16)
    lo = (x - hi.astype(np.float32)).astype(ml_dtypes.bfloat16)
    return hi, lo


def prep_core_inputs(pred_logits, pred_boxes, target_boxes, target_labels, core):
    import ml_dtypes

    s0 = core * NLOC
    pl = np.ascontiguousarray(
        pred_logits[s0 : s0 + NLOC].reshape(NLOC * Q, C), dtype=np.float32
    )
    plp = np.full((NLOC, 384, Q), -30.0, np.float32)
    plp[:, :C, :] = pred_logits[s0 : s0 + NLOC].transpose(0, 2, 1)  # [s, c, q]
    pl_q = np.ascontiguousarray(
        plp.reshape(NPAIR, 2, 3, 128, 2, Q // 2).transpose(0, 2, 1, 4, 3, 5)
    ).astype(ml_dtypes.float8_e4m3)  # [sp, cc, si, qh, ci, l]
    l0h = np.ascontiguousarray(plp[:, 0, :])
    # block-diagonal K=12 matmul operands
    mmrhs = np.zeros((NPAIR, 2, 12, Q, 2), ml_dtypes.bfloat16)
    mmlhs = np.zeros((NPAIR, 2, 12, P2), ml_dtypes.bfloat16)
    r6 = np.zeros((6, 4, Q), np.float32)
    l6 = np.zeros((6, 4, P2), np.float32)
    for p in range(NPAIR):
        a, b = s0 + 2 * p, s0 + 2 * p + 1
        pa_hi, pa_lo = _bf16_split(pred_boxes[a].T)
        pb_hi, pb_lo = _bf16_split(pred_boxes[b].T)
        ta_hi, ta_lo = _bf16_split(target_boxes[a].T)
        tb_hi, tb_lo = _bf16_split(target_boxes[b].T)
        r6[0] = pa_hi
        r6[1] = pa_lo
        r6[2] = -1.0
        r6[3] = -1.0
        r6[4] = pb_hi
        r6[5] = pb_lo
        l6[:] = 0.0
        l6[0, :, :T] = 1.0
        l6[1, :, :T] = 1.0
        l6[2, :, :T] = ta_hi
        l6[3, :, :T] = ta_lo
        l6[2, :, T:] = tb_hi
        l6[3, :, T:] = tb_lo
        l6[4, :, T:] = 1.0
        l6[5, :, T:] = 1.0
        for h in range(2):
            for dd in range(2):
                d = 2 * h + dd
                mmrhs[p, h, dd * 6 : dd * 6 + 6, :, dd] = r6[:, d, :]
                mmlhs[p, h, dd * 6 : dd * 6 + 6, :] = l6[:, d, :]
    labels = target_labels[s0 : s0 + NLOC].astype(np.float32)
    m = {
        "logits": pl,
        "logits_q": pl_q,
        "l0h": l0h,
        "mmrhs": mmrhs,
        "mmlhs": mmlhs,
        "labels": labels,
    }
    m.update(_consts())
    return m


def finalize(loss16_list):
    losses = np.concatenate(
        [np.asarray(l16, np.float32).T.reshape(-1) for l16 in loss16_list]
    )
    return np.float32(losses.mean(dtype=np.float64))


def kernel(pred_logits, pred_boxes, target_boxes, target_labels):
    from concourse.bass_utils import run_bass_kernel_spmd

    pred_logits = np.asarray(pred_logits)
    pred_boxes = np.asarray(pred_boxes)
    target_boxes = np.asarray(target_boxes)
    target_labels = np.asarray(target_labels)

    nc = get_nc()
    in_maps = [
        prep_core_inputs(pred_logits, pred_boxes, target_boxes, target_labels, c)
        for c in range(NCORES)
    ]
    res = run_bass_kernel_spmd(nc, in_maps, core_ids=list(range(NCORES)))
    return finalize([res.results[c]["loss16"] for c in range(NCORES)])


# revision 23
# speedup vs baseline: 1.0242x; 1.0242x over previous
"""DinoV2 detection loss on 8 Trainium2 NeuronCores (Bass/Tile).

Reference computation (per batch sample b; B=128, Q=2048, C=365, T=50):
  dist[q, t] = sum_d |pred_boxes[b,q,d] - target_boxes[b,t,d]|
  closest[t] = argmin_q dist[q, t]
  class_targets = scatter(zeros(Q), closest, labels)     (last write wins)
  loss_ce  = weighted CE over all Q rows (background cls 0 weight 0.1)
  loss_bbox = mean_t,d |pred_boxes[closest[t]] - target_boxes[t]|
  out = mean_b(2*loss_ce + 5*loss_bbox)

Sharding: data-parallel over B; each core handles 16 samples and emits
16 per-sample losses; host averages 128 values.

Per-core device algorithm (samples processed in 8 pairs of 2, laid out on
100 partitions = 2 x 50 targets):
  - Main CE pass over a host-transposed bf16 copy of the logits
    [sample, class, query]: ACT exponentiates two samples per op
    ([128, 4096] per op, bf16 out), PE reduces classes via per-sample
    selector-column matmuls that accumulate each sample's row sums into
    a single persistent PSUM tile [16, 4x512] (f32), ACT takes Ln
    directly from PSUM at the end -> row LSE for all 16 samples at once.
    S_b = sum_q (LSE - logit[...,0]) via one DVE subtract + reduce.
  - Distances: PE matmul trick gives diff[t,(q,d)] = pb[q,d] - tb[t,d]
    (contraction over indicator/value rows; boxes split hi/lo into two
    bf16 components for near-fp32 accuracy; two block-diagonal K=12
    matmuls per 256-query chunk cover all 4 box dims), DVE reduces
    |diff| over d via one XY-axis abs-add reduce per chunk, then
    reduce-min + max_index give (min dist, argmin).
  - Duplicate-match resolution ("last write wins") via an indirect-DMA
    scatter of the target index to dram slot [matched row] followed by a
    gather readback on the same FIFO DMA queue; mask = (readback == t).
  - Matched corrections: indirect-DMA gather of the 50 matched logit
    rows per sample from the row-major f32 logits, exp+accum for their
    LSE; the target-class logit comes from a second single-element
    indirect gather at flat offset row*365+label.
  - GPSIMD (Pool) engine runs all small glue ops (casts, broadcasts,
    masks) to keep DVE on the distance-reduction critical path.
"""

import numpy as np

B, Q, C, T = 128, 2048, 365, 50
NCORES = 8
NLOC = B // NCORES          # 16 samples per core
NPAIR = NLOC // 2           # 8 pairs
P2 = 2 * T                  # 100 partitions per pair tile
QCH = 256                   # dist matmul q-chunk (x2 box dims = 512 cols)
NQC = Q // QCH              # 8
W_BG = float(np.float32(0.1))
DEN0 = float(np.float32(0.1) * 2048)   # background weight sum

_CACHE = {}


def _build_nc():
    import concourse.bacc as bacc
    import concourse.bass as bass
    import concourse.mybir as mybir
    import concourse.tile as tile

    f32 = mybir.dt.float32
    bf16 = mybir.dt.bfloat16
    i32 = mybir.dt.int32
    Alu = mybir.AluOpType
    Act = mybir.ActivationFunctionType
    Ax = mybir.AxisListType

    nc = bacc.Bacc("TRN2", target_bir_lowering=False, debug=False)

    # row-major f32 logits: read by the matched-row / matched-element gathers
    logits = nc.dram_tensor("logits", [NLOC * Q, C], f32, kind="ExternalInput")
    # transposed bf16 logits for the bulk CE pass, repacked as
    # [sample, class-chunk, q-half, class-in-chunk, q-in-half]: each
    # (sample, chunk) block is one contiguous 512KB region whose DMA
    # partition stride is 2KB -- both properties are needed for the
    # descriptor splitter to spread the transfer across all 16 SDMA
    # engines. Classes padded 365->384 with -30 (exp ~ 0).
    f8 = mybir.dt.float8e4
    logits_q = nc.dram_tensor(
        "logits_q", [NPAIR, 3, 2, 2, 128, Q // 2], f8, kind="ExternalInput"
    )
    l0h = nc.dram_tensor("l0h", [NLOC, Q], f32, kind="ExternalInput")
    # block-diagonal hi/lo bf16 operands of the box-diff matmuls:
    # [pair, dim-pair h, 12 rows, Q, 2]: two K=12 matmuls per q-chunk
    # produce diff for d = 2h+dd at psum col (q, dd).
    mmrhs = nc.dram_tensor("mmrhs", [NPAIR, 2, 12, Q, 2], bf16, kind="ExternalInput")
    mmlhs = nc.dram_tensor("mmlhs", [NPAIR, 2, 12, P2], bf16, kind="ExternalInput")
    labels = nc.dram_tensor("labels", [NLOC, T], f32, kind="ExternalInput")
    # esel[c, s, j] = (j == s): selector columns for per-sample row sums
    esel = nc.dram_tensor("esel", [128, 16, 16], bf16, kind="ExternalInput")
    halfoff = nc.dram_tensor("halfoff", [P2, 1], f32, kind="ExternalInput")
    tvec = nc.dram_tensor("tvec", [P2, 1], f32, kind="ExternalInput")
    blockhalf = nc.dram_tensor("blockhalf", [P2, 2], f32, kind="ExternalInput")
    # scratch for scatter/readback duplicate detection
    dupbuf = nc.dram_tensor("dupbuf", [NLOC * Q, 1], f32, kind="Internal")
    loss16 = nc.dram_tensor("loss16", [2, NPAIR], f32, kind="ExternalOutput")

    with tile.TileContext(nc) as tc:
        with (
            tc.tile_pool(name="const", bufs=1) as cpool,
            tc.tile_pool(name="logits", bufs=3) as lpool,
            tc.tile_pool(name="expbf", bufs=3) as epool,
            tc.tile_pool(name="scr", bufs=2) as spool,
            tc.tile_pool(name="acc", bufs=1) as apool,
            tc.tile_pool(name="pair", bufs=3) as ppool,
            tc.tile_pool(name="dram", bufs=1, space="DRAM") as dpool,
            tc.tile_pool(name="psd", bufs=2, space="PSUM") as psd,
            tc.tile_pool(name="ceps", bufs=1, space="PSUM") as pce,
        ):
            # ---- prefetch pair-0 box operands, then the first sample-pair ----
            def emit_pair_dma(p):
                rhs_t = ppool.tile([12, 2, Q, 2], bf16, tag="rhs_t")
                qm = Q // 2
                nc.sync.dma_start(out=rhs_t[:, 0, :qm], in_=mmrhs.ap()[p, 0, :, :qm])
                nc.gpsimd.dma_start(out=rhs_t[:, 1, :qm], in_=mmrhs.ap()[p, 1, :, :qm])
                nc.sync.dma_start(out=rhs_t[:, 0, qm:], in_=mmrhs.ap()[p, 0, :, qm:])
                nc.gpsimd.dma_start(out=rhs_t[:, 1, qm:], in_=mmrhs.ap()[p, 1, :, qm:])
                lhs_t = ppool.tile([12, 2, P2], bf16, tag="lhs_t")
                nc.gpsimd.dma_start(
                    out=lhs_t[:],
                    in_=mmlhs.ap()[p, :, :, :].rearrange("h k t -> k h t"),
                )
                dist = ppool.tile([P2, Q], f32, tag="dist")
                return rhs_t, lhs_t, dist

            pair_tiles = {0: emit_pair_dma(0)}
            ch_tiles = {}
            for cc in range(3):
                ch = lpool.tile([128, 2, 2, Q // 2], f8, tag="chunk")
                for si in range(2):
                    nc.sync.dma_start(
                        out=ch[:, si, :, :],
                        in_=logits_q.ap()[0, cc, si].rearrange("qh c l -> c qh l"),
                    )
                ch_tiles[(0, cc)] = ch
            esel_sb = cpool.tile([128, 16, 16], bf16, tag="esel")
            nc.sync.dma_start(out=esel_sb[:], in_=esel.ap())

            # ---- constants into SBUF ----
            hoff_sb = cpool.tile([P2, 1], f32, tag="hoff")
            nc.gpsimd.dma_start(out=hoff_sb[:], in_=halfoff.ap())
            tvec_sb = cpool.tile([P2, 1], f32, tag="tvec")
            nc.gpsimd.dma_start(out=tvec_sb[:], in_=tvec.ap())
            # labels -> [100, 8]: partition (h*50+t), col p holds labels[2p+h, t]
            lab_sb = cpool.tile([P2, NPAIR], f32, tag="lab")
            lab_src = bass.AP(
                tensor=labels, offset=0, ap=[[T, 2], [1, T], [2 * T, NPAIR]]
            )
            nc.gpsimd.dma_start(out=lab_sb[:], in_=lab_src)
            bh_sb = cpool.tile([P2, 2], f32, tag="bh")
            nc.gpsimd.dma_start(out=bh_sb[:], in_=blockhalf.ap())
            # zero-fill the duplicate-detection scratch once (on the same
            # FIFO queue as the later scatters)
            zf = cpool.tile([128, NLOC * Q // 128], f32, tag="zf")
            nc.gpsimd.memset(zf[:], 0.0)
            nc.gpsimd.dma_start(
                out=dupbuf.ap().rearrange("(a b) o -> a (b o)", a=128),
                in_=zf[:],
            )

            # ---- accumulators ----
            l0_all = apool.tile([NLOC, Q], f32, tag="l0")
            lse_all = apool.tile([NLOC, Q], f32, tag="lse")
            mind_all = apool.tile([P2, NPAIR], f32, tag="mind")
            mask_all = apool.tile([P2, NPAIR], f32, tag="mask")
            sume_all = apool.tile([P2, NPAIR], f32, tag="sume")
            ly_all = apool.tile([P2, NPAIR], f32, tag="ly")
            l0m_all = apool.tile([P2, NPAIR], f32, tag="l0m")
            rb_all = apool.tile([P2, NPAIR], f32, tag="rball")

            # persistent CE row-sum accumulator: bank g holds queries
            # [512g, 512(g+1)) for all 16 samples
            ce_ps = pce.tile([NLOC, 4, 512], f32, tag="ceps")

            def emit_pair_dma(p):
                rhs_t = ppool.tile([12, 2, Q, 2], bf16, tag="rhs_t")
                qm = Q // 2
                nc.sync.dma_start(out=rhs_t[:, 0, :qm], in_=mmrhs.ap()[p, 0, :, :qm])
                nc.gpsimd.dma_start(out=rhs_t[:, 1, :qm], in_=mmrhs.ap()[p, 1, :, :qm])
                nc.sync.dma_start(out=rhs_t[:, 0, qm:], in_=mmrhs.ap()[p, 0, :, qm:])
                nc.gpsimd.dma_start(out=rhs_t[:, 1, qm:], in_=mmrhs.ap()[p, 1, :, qm:])
                lhs_t = ppool.tile([12, 2, P2], bf16, tag="lhs_t")
                nc.gpsimd.dma_start(
                    out=lhs_t[:],
                    in_=mmlhs.ap()[p, :, :, :].rearrange("h k t -> k h t"),
                )
                dist = ppool.tile([P2, Q], f32, tag="dist")
                return rhs_t, lhs_t, dist

            def emit_pair_chunk(pt, qc):
                rhs_t, lhs_t, dist = pt
                ps = psd.tile([P2, 2, QCH, 2], f32, tag="psd")
                for h in range(2):
                    nc.tensor.matmul(
                        out=ps[:, h, :, :],
                        lhsT=lhs_t[:, h, :],
                        rhs=rhs_t[:, h, qc * QCH : (qc + 1) * QCH, :],
                        start=True,
                        stop=True,
                    )
                nc.vector.tensor_reduce(
                    out=dist[:, qc * QCH : (qc + 1) * QCH],
                    in_=ps[:].rearrange("p h q d -> p q h d"),
                    axis=Ax.XY,
                    op=Alu.add,
                    apply_absolute_value=True,
                )

            def emit_spair_cc(sp, cc):
                # two samples (2sp, 2sp+1) per exp op; selector-column
                # matmuls accumulate each sample's class sums into ce_ps
                if (sp, cc) in ch_tiles:
                    ch = ch_tiles.pop((sp, cc))
                else:
                    ch = lpool.tile([128, 2, 2, Q // 2], f8, tag="chunk")
                    for si in range(2):
                        nc.sync.dma_start(
                            out=ch[:, si, :, :],
                            in_=logits_q.ap()[sp, cc, si].rearrange(
                                "qh c l -> c qh l"
                            ),
                        )
                eb = epool.tile([128, 2, 2, Q // 2], bf16, tag="expbf")
                nc.scalar.activation(eb[:], ch[:], Act.Exp)
                for si in range(2):
                    s = 2 * sp + si
                    for g in range(4):
                        qh, lh = g // 2, g % 2
                        nc.tensor.matmul(
                            out=ce_ps[:, g, :],
                            lhsT=esel_sb[:, s, :],
                            rhs=eb[:, si, qh, lh * 512 : (lh + 1) * 512],
                            start=(sp == 0 and cc == 0 and si == 0),
                            stop=(sp == NPAIR - 1 and cc == 2 and si == 1),
                        )

            def emit_pair_finalize(p, pt):
                _, _, dist = pt
                nc.vector.tensor_reduce(
                    out=mind_all[:, p : p + 1], in_=dist[:], axis=Ax.X, op=Alu.min
                )
                mind8 = ppool.tile([P2, 8], f32, tag="mind8")
                nc.vector.tensor_copy(
                    out=mind8[:], in_=mind_all[:, p : p + 1].to_broadcast([P2, 8])
                )
                idxu = ppool.tile([P2, 8], mybir.dt.uint32, tag="idxu")
                nc.vector.max_index(out=idxu[:], in_max=mind8[:], in_values=dist[:])
                idxf = ppool.tile([P2, 1], f32, tag="idxf")
                nc.gpsimd.tensor_copy(out=idxf[:], in_=idxu[:, 0:1])
                rowf = ppool.tile([P2, 1], f32, tag="rowf")
                nc.gpsimd.tensor_scalar(
                    rowf[:],
                    idxf[:],
                    hoff_sb[:],
                    float(p * 2 * Q),
                    op0=Alu.add,
                    op1=Alu.add,
                )
                rowi = ppool.tile([P2, 1], i32, tag="rowi")
                nc.gpsimd.tensor_copy(out=rowi[:], in_=rowf[:])
                # flat offset of the target-class logit: row*365 + label
                rowlyf = ppool.tile([P2, 1], f32, tag="rowlyf")
                nc.gpsimd.tensor_scalar(
                    rowlyf[:],
                    rowf[:],
                    float(C),
                    lab_sb[:, p : p + 1],
                    op0=Alu.mult,
                    op1=Alu.add,
                )
                rowlyi = ppool.tile([P2, 1], i32, tag="rowlyi")
                nc.gpsimd.tensor_copy(out=rowlyi[:], in_=rowlyf[:])

                # duplicate detection: scatter t -> dupbuf[row[t]] then read
                # back on the same FIFO queue; last write wins, matching the
                # reference scatter semantics.
                nc.gpsimd.indirect_dma_start(
                    out=dupbuf.ap(),
                    out_offset=bass.IndirectOffsetOnAxis(ap=rowi[:, 0:1], axis=0),
                    in_=tvec_sb[:, 0:1],
                    in_offset=None,
                )
                nc.gpsimd.indirect_dma_start(
                    out=rb_all[:, p : p + 1],
                    out_offset=None,
                    in_=dupbuf.ap(),
                    in_offset=bass.IndirectOffsetOnAxis(ap=rowi[:, 0:1], axis=0),
                )

                # gather matched logit rows (row-major f32 copy)
                rows_sb = ppool.tile([P2, C], f32, tag="rows")
                nc.gpsimd.indirect_dma_start(
                    out=rows_sb[:],
                    out_offset=None,
                    in_=logits.ap(),
                    in_offset=bass.IndirectOffsetOnAxis(ap=rowi[:, 0:1], axis=0),
                )
                # target-class logit via single-element gather on flat view
                flat = bass.AP(tensor=logits, offset=0, ap=[[1, NLOC * Q * C], [1, 1]])
                nc.gpsimd.indirect_dma_start(
                    out=ly_all[:, p : p + 1],
                    out_offset=None,
                    in_=flat,
                    in_offset=bass.IndirectOffsetOnAxis(ap=rowlyi[:, 0:1], axis=0),
                )
                nc.gpsimd.tensor_copy(
                    out=l0m_all[:, p : p + 1], in_=rows_sb[:, 0:1]
                )
                return rows_sb

            def emit_matched(p, rows_sb):
                scr2 = spool.tile([P2, C], f32, tag="expdump")
                nc.scalar.activation(
                    scr2[:],
                    rows_sb[:],
                    Act.Exp,
                    accum_out=sume_all[:, p : p + 1],
                )

            # main pass: pair p's box-diff chunks interleave with sample-pair
            # p's CE work (dist chunks lead each cc section so PE has ready
            # work while exp cooks); pair DMAs prefetch one spair ahead;
            # matched-row work trails by one pair.
            rows_tiles = {}
            # l0 loads after the critical head DMAs
            nc.gpsimd.dma_start(out=l0_all[:], in_=l0h.ap())
            CHSPLIT = [(0, 4), (4, 7), (7, 8)]
            for sp in range(NPAIR):
                if sp + 1 < NPAIR:
                    pair_tiles[sp + 1] = emit_pair_dma(sp + 1)
                pt = pair_tiles[sp]
                for cc in range(3):
                    for qc in range(*CHSPLIT[cc]):
                        emit_pair_chunk(pt, qc)
                    emit_spair_cc(sp, cc)
                rows_tiles[sp] = emit_pair_finalize(sp, pt)
                if sp > 0:
                    emit_matched(sp - 1, rows_tiles[sp - 1])
                if sp == NPAIR - 1:
                    emit_matched(sp, rows_tiles[sp])

            # ---- main CE reduction: S_b = sum_q (LSE - l0) ----
            nc.scalar.activation(
                lse_all[:],
                ce_ps[:].rearrange("s g q -> s (g q)"),
                Act.Ln,
            )
            diff = apool.tile([NLOC, Q], f32, tag="diff")
            nc.vector.tensor_sub(diff[:], lse_all[:], l0_all[:])
            s16 = apool.tile([NLOC, 1], f32, tag="s16")
            nc.vector.tensor_reduce(
                out=s16[:], in_=diff[:], axis=Ax.X, op=Alu.add
            )
            # [16,1] -> [2,8] via DRAM bounce: s = 2p + h
            s16d = dpool.tile([1, NLOC], f32, tag="s16d")
            nc.gpsimd.dma_start(out=s16d[:], in_=s16[:])
            s2 = apool.tile([2, NPAIR], f32, tag="s2")
            nc.gpsimd.dma_start(
                out=s2[:], in_=s16d[:].rearrange("o (pp h) -> o h pp", h=2)
            )

            # ---- matched-term assembly ----
            nc.vector.tensor_tensor(
                out=mask_all[:],
                in0=rb_all[:],
                in1=tvec_sb[:].to_broadcast([P2, NPAIR]),
                op=Alu.is_equal,
            )
            lsem = apool.tile([P2, NPAIR], f32, tag="lsem")
            nc.scalar.activation(lsem[:], sume_all[:], Act.Ln)
            wy = apool.tile([P2, NPAIR], f32, tag="wy")
            # wy = 1 - 0.9*(label==0)
            nc.vector.tensor_scalar(
                wy[:], lab_sb[:], 0.0, None, op0=Alu.is_equal
            )
            nc.gpsimd.tensor_scalar(
                wy[:], wy[:], -(1.0 - W_BG), 1.0, op0=Alu.mult, op1=Alu.add
            )
            nllm = apool.tile([P2, NPAIR], f32, tag="nllm")
            nc.vector.tensor_sub(nllm[:], lsem[:], ly_all[:])
            stack3 = apool.tile([P2, 3 * NPAIR], f32, tag="stack3")
            corr = stack3[:, 0:NPAIR]
            nc.vector.tensor_mul(corr, wy[:], nllm[:])
            t2 = apool.tile([P2, NPAIR], f32, tag="t2")
            nc.gpsimd.tensor_scalar(
                t2[:], lsem[:], -W_BG, None, op0=Alu.mult
            )
            nc.vector.tensor_add(corr, corr, t2[:])
            nc.gpsimd.tensor_scalar(
                t2[:], l0m_all[:], W_BG, None, op0=Alu.mult
            )
            nc.vector.tensor_add(corr, corr, t2[:])
            nc.vector.tensor_mul(corr, corr, mask_all[:])
            wadd = stack3[:, NPAIR : 2 * NPAIR]
            nc.gpsimd.tensor_scalar(
                wadd, wy[:], -W_BG, None, op0=Alu.add
            )
            nc.vector.tensor_mul(wadd, wadd, mask_all[:])
            nc.gpsimd.tensor_copy(out=stack3[:, 2 * NPAIR :], in_=mind_all[:])

            # final per-sample combine: reuse bank 0 of the (now read) CE
            # psum tile for the [2, 24] block-sum matmul
            ps_c = ce_ps[0:2, 0, 0:24]
            nc.tensor.matmul(
                out=ps_c, lhsT=bh_sb[:], rhs=stack3[:], start=True, stop=True
            )
            num = apool.tile([2, NPAIR], f32, tag="num")
            nc.vector.tensor_scalar(num[:], s2[:], W_BG, None, op0=Alu.mult)
            nc.vector.tensor_add(num[:], num[:], ps_c[:, 0:NPAIR])
            den = apool.tile([2, NPAIR], f32, tag="den")
            nc.vector.tensor_scalar(
                den[:], ps_c[:, NPAIR : 2 * NPAIR], DEN0, None, op0=Alu.add
            )
            rden = apool.tile([2, NPAIR], f32, tag="rden")
            nc.vector.reciprocal(rden[:], den[:])
            lce = apool.tile([2, NPAIR], f32, tag="lce")
            nc.vector.tensor_mul(lce[:], num[:], rden[:])
            nc.vector.tensor_scalar(lce[:], lce[:], 2.0, None, op0=Alu.mult)
            bbox = apool.tile([2, NPAIR], f32, tag="bbox")
            nc.vector.tensor_scalar(
                bbox[:], ps_c[:, 2 * NPAIR :], 5.0 / (T * 4), None, op0=Alu.mult
            )
            out_sb = apool.tile([2, NPAIR], f32, tag="out")
            nc.vector.tensor_add(out_sb[:], lce[:], bbox[:])
            nc.sync.dma_start(out=loss16.ap(), in_=out_sb[:])

    nc.compile()
    return nc


def get_nc():
    if "nc" not in _CACHE:
        _CACHE["nc"] = _build_nc()
    return _CACHE["nc"]


def _consts():
    import ml_dtypes

    halfoff = ((np.arange(P2) >= T) * Q).astype(np.float32)[:, None]
    tvec = np.arange(P2, dtype=np.float32)[:, None]
    esel = np.zeros((128, 16, 16), ml_dtypes.bfloat16)
    for s in range(16):
        esel[:, s, s] = 1.0
    blockhalf = np.zeros((P2, 2), np.float32)
    blockhalf[:T, 0] = 1.0
    blockhalf[T:, 1] = 1.0
    return {
        "halfoff": halfoff,
        "tvec": tvec,
        "esel": esel,
        "blockhalf": blockhalf,
    }


def _bf16_split(x):
    import ml_dtypes

    hi = x.astype(ml_dtypes.bfloat16)
    lo = (x - hi.astype(np.float32)).astype(ml_dtypes.bfloat16)
    return hi, lo


def prep_core_inputs(pred_logits, pred_boxes, target_boxes, target_labels, core):
    import ml_dtypes

    s0 = core * NLOC
    pl = np.ascontiguousarray(
        pred_logits[s0 : s0 + NLOC].reshape(NLOC * Q, C), dtype=np.float32
    )
    plp = np.full((NLOC, 384, Q), -30.0, np.float32)
    plp[:, :C, :] = pred_logits[s0 : s0 + NLOC].transpose(0, 2, 1)  # [s, c, q]
    pl_q = np.ascontiguousarray(
        plp.reshape(NPAIR, 2, 3, 128, 2, Q // 2).transpose(0, 2, 1, 4, 3, 5)
    ).astype(ml_dtypes.float8_e4m3)  # [sp, cc, si, qh, ci, l]
    l0h = np.ascontiguousarray(plp[:, 0, :])
    # block-diagonal K=12 matmul operands
    mmrhs = np.zeros((NPAIR, 2, 12, Q, 2), ml_dtypes.bfloat16)
    mmlhs = np.zeros((NPAIR, 2, 12, P2), ml_dtypes.bfloat16)
    r6 = np.zeros((6, 4, Q), np.float32)
    l6 = np.zeros((6, 4, P2), np.float32)
    for p in range(NPAIR):
        a, b = s0 + 2 * p, s0 + 2 * p + 1
        pa_hi, pa_lo = _bf16_split(pred_boxes[a].T)
        pb_hi, pb_lo = _bf16_split(pred_boxes[b].T)
        ta_hi, ta_lo = _bf16_split(target_boxes[a].T)
        tb_hi, tb_lo = _bf16_split(target_boxes[b].T)
        r6[0] = pa_hi
        r6[1] = pa_lo
        r6[2] = -1.0
        r6[3] = -1.0
        r6[4] = pb_hi
        r6[5] = pb_lo
        l6[:] = 0.0
        l6[0, :, :T] = 1.0
        l6[1, :, :T] = 1.0
        l6[2, :, :T] = ta_hi
        l6[3, :, :T] = ta_lo
        l6[2, :, T:] = tb_hi
        l6[3, :, T:] = tb_lo
        l6[4, :, T:] = 1.0
        l6[5, :, T:] = 1.0
        for h in range(2):
            for dd in range(2):
                d = 2 * h + dd
                mmrhs[p, h, dd * 6 : dd * 6 + 6, :, dd] = r6[:, d, :]
                mmlhs[p, h, dd * 6 : dd * 6 + 6, :] = l6[:, d, :]
    labels = target_labels[s0 : s0 + NLOC].astype(np.float32)
    m = {
        "logits": pl,
        "logits_q": pl_q,
        "l0h": l0h,
        "mmrhs": mmrhs,
        "mmlhs": mmlhs,
        "labels": labels,
    }
    m.update(_consts())
    return m


def finalize(loss16_list):
    losses = np.concatenate(
        [np.asarray(l16, np.float32).T.reshape(-1) for l16 in loss16_list]
    )
    return np.float32(losses.mean(dtype=np.float64))


def kernel(pred_logits, pred_boxes, target_boxes, target_labels):
    from concourse.bass_utils import run_bass_kernel_spmd

    pred_logits = np.asarray(pred_logits)
    pred_boxes = np.asarray(pred_boxes)
    target_boxes = np.asarray(target_boxes)
    target_labels = np.asarray(target_labels)

    nc = get_nc()
    in_maps = [
        prep_core_inputs(pred_logits, pred_boxes, target_boxes, target_labels, c)
        for c in range(NCORES)
    ]
    res = run_bass_kernel_spmd(nc, in_maps, core_ids=list(range(NCORES)))
    return finalize([res.results[c]["loss16"] for c in range(NCORES)])
